# revision 30
# baseline (speedup 1.0000x reference)
"""Trainium2 Bass kernel for the BlockDiagonalACDC layer (parity-split L2).

out = riffle(idct2(gconv(dct2(gconv(x, A)), D))) + bias. Two levels of
DCT-II parity symmetry:
  L1: u+/u- = z1[:2048] +- rev(z1[2048:]); O-side (odd k) stays dense 2048.
  L2: v+/v- = u+[:1024] +- rev(u+[1024:]); z2_Ee = DCTII_1024(v+),
      z2_Eo = DCTIV_1024(v-). Dense fwd work drops 8M->6M MACs/row.
  gconv(D) conjugated into the (Ee, Eo, O) basis via 9 packed lhsT tiles
      built on device from D.
  inv:  sa = z3_Ee @ GmA, sb = z3_Eo @ GmB shared between PAIRED output
      strips (s_direct = sa+sb, s_rev = rev(sa-sb)); t = z3_O @ GmO per
      strip. Bias enters as K=1 rows on all four PSUM chains.

Sharding: pure data parallel, 2048 batch rows per core on 8 cores.
"""

import numpy as np
import ml_dtypes

import concourse.bacc as bacc
import concourse.mybir as mybir
from concourse.tile import TileContext
from concourse.bass_utils import run_bass_kernel_spmd
from concourse.masks import make_identity

N_BATCH, D_FEAT, GROUPS = 16384, 4096, 32
N_CORES = 8
N_SHARD = N_BATCH // N_CORES      # 2048 rows per core
CHUNK = 512                       # batch rows per pipeline chunk
N_CHUNKS = N_SHARD // CHUNK       # 4
FTILES = D_FEAT // 128            # 32
HT = FTILES // 2                  # 16
HM = 1024                         # level-2 half length
QW = 256                          # inverse strip width
NQ = 2048 // QW                   # 8 strips

_BF16 = mybir.dt.bfloat16
_F32 = mybir.dt.float32

PLUS_START = [256 * c if c < 4 else 1024 + 256 * c for c in range(8)]
MINUS_LO = [3840 - 256 * c if c < 4 else 2816 - 256 * c for c in range(8)]
PAIRS = [(0, 7), (1, 6), (4, 3), (5, 2)]


# AT slot map: normal g<24 at g; reversed g in 8..15 -> 24+(g-8),
# g in 24..31 -> 32+(g-24); reversed-negated g in 16..31 -> 40+(g-16)
def _at_n(g):
    assert g < 24
    return g


def _at_r(g):
    return 24 + (g - 8) if g < 16 else 32 + (g - 24)


def _at_rn(g):
    return 40 + (g - 16)


AT_SLOTS = 56


def _host_constants():
    N = D_FEAT
    H = N // 2
    m = HM
    j = np.arange(N, dtype=np.float64)
    k = np.arange(N, dtype=np.float64)[:, None]
    ang = np.pi * k * (2.0 * j[None, :] + 1.0) / (2.0 * N)
    C = 2.0 * np.cos(ang)
    w = np.ones(N); w[0] = 0.5
    Gm = (1.0 / N) * w[:, None] * np.cos(ang)         # [k, j]

    rho = np.arange(m, dtype=np.float64)[:, None]
    jp = np.arange(m, dtype=np.float64)[None, :]
    C2p = 2.0 * np.cos(np.pi * rho * (2 * jp + 1) / (2 * m))
    C4p = 2.0 * np.cos(np.pi * (2 * rho + 1) * (2 * jp + 1) / (4 * m))
    O1 = C[1::2, :H].T                                # [j, r]

    fwdw_e = np.zeros((2, 8, 128, 8, 128))
    for side, M2 in ((0, C2p), (1, C4p)):
        for c in range(8):
            for kc in range(8):
                fwdw_e[side, c, :, kc, :] = \
                    M2[128*c:128*c+128, 128*kc:128*kc+128].T
    fwdw_o = np.zeros((HT, 128, HT, 128))
    for tau in range(HT):
        for fc in range(HT):
            fwdw_o[tau, :, fc, :] = O1[128*fc:128*fc+128, 128*tau:128*tau+128]

    cols = np.concatenate([np.arange(0, H, 2), np.arange(1, H, 2)])
    GmA = Gm[0::4, :m]
    GmB = Gm[2::4, :m]
    GmO = Gm[1::2, :H]
    invw_e = np.zeros((4, 2, 128, 8, QW))
    for pi, (cd, cr) in enumerate(PAIRS):
        jset = cols[np.arange(QW*cd, QW*cd+QW)]
        for kc in range(8):
            invw_e[pi, 0, :, kc, :] = GmA[128*kc:128*kc+128][:, jset]
            invw_e[pi, 1, :, kc, :] = GmB[128*kc:128*kc+128][:, jset]
    Gq = GmO[:, cols]
    invw_o = np.zeros((NQ, 128, HT, QW))
    for c in range(NQ):
        for kc in range(HT):
            invw_o[c, :, kc, :] = Gq[128*kc:128*kc+128, QW*c:QW*c+QW]

    out_plus = np.where(cols % 2 == 0, cols // 2, 2048 + (cols - 1) // 2)
    jm = 4095 - cols
    out_minus = np.where(jm % 2 == 0, jm // 2, 2048 + (jm - 1) // 2)

    return (fwdw_o.astype(ml_dtypes.bfloat16),
            fwdw_e.astype(ml_dtypes.bfloat16),
            invw_o.astype(ml_dtypes.bfloat16),
            invw_e.astype(ml_dtypes.bfloat16),
            out_plus.astype(np.int64), out_minus.astype(np.int64))


def _host_bias(bias, out_plus, out_minus):
    bias_v = np.asarray(bias, dtype=np.float64).reshape(-1)
    bs = (bias_v[out_plus] + bias_v[out_minus]) / 2
    bt = (bias_v[out_plus] - bias_v[out_minus]) / 2
    bias_sa = np.zeros(4 * QW)
    bias_sb = np.zeros(4 * QW)
    for pi, (cd, cr) in enumerate(PAIRS):
        S = bs[QW*cd:QW*cd+QW]
        Dl = bs[QW*cr:QW*cr+QW][::-1]
        bias_sa[QW*pi:QW*pi+QW] = (S + Dl) / 2
        bias_sb[QW*pi:QW*pi+QW] = (S - Dl) / 2
    return (bias_sa.astype(np.float32)[None],
            bias_sb.astype(np.float32)[None],
            bt.astype(np.float32)[None])


def _build_program(reps=1, unroll=1):
    nc = bacc.Bacc()
    xs = nc.dram_tensor("xs", (N_SHARD, D_FEAT), _F32, kind="ExternalInput")
    Aw = nc.dram_tensor("Aw", (GROUPS, 128, 128), _F32, kind="ExternalInput")
    Dw = nc.dram_tensor("Dw", (GROUPS, 128, 128), _F32, kind="ExternalInput")
    bias_sa = nc.dram_tensor("bias_sa", (1, 1024), _F32, kind="ExternalInput")
    bias_sb = nc.dram_tensor("bias_sb", (1, 1024), _F32, kind="ExternalInput")
    bias_t = nc.dram_tensor("bias_t", (1, 2048), _F32, kind="ExternalInput")
    fwdw_o = nc.dram_tensor("fwdw_o", (HT, 128, HT, 128), _BF16,
                            kind="ExternalInput")
    fwdw_e = nc.dram_tensor("fwdw_e", (2, 8, 128, 8, 128), _BF16,
                            kind="ExternalInput")
    invw_o = nc.dram_tensor("invw_o", (NQ, 128, HT, QW), _BF16,
                            kind="ExternalInput")
    invw_e = nc.dram_tensor("invw_e", (4, 2, 128, 8, QW), _BF16,
                            kind="ExternalInput")
    out = nc.dram_tensor("out", (N_SHARD, D_FEAT), _F32, kind="ExternalOutput")

    with TileContext(nc) as tc:
        with (
            tc.tile_pool(name="const", bufs=1) as constp,
            tc.tile_pool(name="stage", bufs=3) as stagep,
            tc.tile_pool(name="xbf", bufs=2) as xbfp,
            tc.tile_pool(name="fwp", bufs=4) as fwp,
            tc.tile_pool(name="ivp", bufs=7) as ivp,
            tc.tile_pool(name="ost", bufs=6) as ostp,
            tc.tile_pool(name="r1p", bufs=2) as r1p,
            tc.tile_pool(name="mm_ps", bufs=3, space="PSUM") as mmp,
            tc.tile_pool(name="tp_ps", bufs=2, space="PSUM") as tpp,
            tc.tile_pool(name="st_ps", bufs=3, space="PSUM") as stp,
        ):
            ident = constp.tile([128, 128], _BF16, tag="ident")
            make_identity(nc, ident[:])
            ones1 = constp.tile([1, 128], _BF16, tag="ones1")
            nc.gpsimd.memset(ones1[:], 1.0)
            bsa_bf = constp.tile([1, 1024], _BF16, tag="bsa")
            bsb_bf = constp.tile([1, 1024], _BF16, tag="bsb")
            bt_bf = constp.tile([1, 2048], _BF16, tag="bt")
            nc.gpsimd.dma_start(bsa_bf[:], bias_sa[:])
            nc.gpsimd.dma_start(bsb_bf[:], bias_sb[:])
            nc.gpsimd.dma_start(bt_bf[:], bias_t[:])

            # ---- A weights: 56 oriented slots
            AT = constp.tile([128, AT_SLOTS * 128], _BF16, tag="AT")
            awbf = stagep.tile([128, D_FEAT], _BF16, tag="stage")
            for g in range(GROUPS):
                nc.gpsimd.dma_start(awbf[:, g * 128:(g + 1) * 128], Aw[g])

            def at_sl(s):
                return slice(s * 128, (s + 1) * 128)

            for g4 in range(GROUPS // 4):
                ps = tpp.tile([128, 512], _BF16, tag="tp")
                for gg in range(4):
                    g = g4 * 4 + gg
                    nc.tensor.transpose(
                        ps[:, gg * 128:(gg + 1) * 128],
                        awbf[:, g * 128:(g + 1) * 128], ident[:])
                for gg in range(4):
                    g = g4 * 4 + gg
                    fwd_sl = slice(gg * 128, (gg + 1) * 128)
                    rev_sl = slice((gg + 1) * 128 - 1,
                                   gg * 128 - 1 if gg else None, -1)
                    if g < 24:
                        nc.vector.tensor_copy(AT[:, at_sl(_at_n(g))],
                                              ps[:, fwd_sl])
                    if 8 <= g < 16 or g >= 24:
                        nc.vector.tensor_copy(AT[:, at_sl(_at_r(g))],
                                              ps[:, rev_sl])
                    if g >= 16:
                        # negated reversed for the u- accumulation
                        nc.scalar.mul(AT[:, at_sl(_at_rn(g))],
                                      ps[:, rev_sl], -1.0)

            # ---- D weights: 9 conjugated packs in the (Ee, Eo, O) basis
            dwbf = stagep.tile([128, D_FEAT], _BF16, tag="stage")
            for g in range(GROUPS):
                nc.gpsimd.dma_start(dwbf[:, g * 128:(g + 1) * 128], Dw[g])
            Wee = constp.tile([128, 1024], _BF16, tag="Wee")
            Weo = constp.tile([128, 1024], _BF16, tag="Weo")
            Woe = constp.tile([128, 1024], _BF16, tag="Woe")
            Woo = constp.tile([128, 1024], _BF16, tag="Woo")
            WeO = constp.tile([128, 2048], _BF16, tag="WeO")
            WoO = constp.tile([128, 2048], _BF16, tag="WoO")
            WOe = constp.tile([128, 2048], _BF16, tag="WOe")
            WOo = constp.tile([128, 2048], _BF16, tag="WOo")
            WOO = constp.tile([128, 2048], _BF16, tag="WOO")
            for wt in (Wee, Weo, Woe, Woo, WeO, WoO, WOe, WOo, WOO):
                nc.gpsimd.memset(wt[:], 0.0)
            for g in range(GROUPS):
                c, gg = g // 4, g % 4
                gb = g * 128
                ps = tpp.tile([128, 512], _BF16, tag="tp")
                # T_Ee [32p,128ko] | T_Eo [32p,128ko] | T_O [64p,128ko]
                nc.tensor.transpose(ps[0:32, 0:128], dwbf[:, gb:gb + 128:4],
                                    ident[:])
                nc.tensor.transpose(ps[0:32, 128:256],
                                    dwbf[:, gb + 2:gb + 128:4], ident[:])
                nc.tensor.transpose(ps[0:64, 256:384],
                                    dwbf[:, gb + 1:gb + 128:2], ident[:])
                pr = slice(32 * gg, 32 * gg + 32)
                po = slice(64 * (g % 2), 64 * (g % 2) + 64)
                cE = slice(128 * c + 32 * gg, 128 * c + 32 * gg + 32)
                h = gg // 2
                cEO = slice(128 * (2 * c + h) + 32 * gg,
                            128 * (2 * c + h) + 32 * gg + 32)
                to = g // 2
                cO = slice(128 * to + 64 * (g % 2),
                           128 * to + 64 * (g % 2) + 64)
                nc.vector.tensor_copy(Wee[pr, cE], ps[0:32, 0:128:4])
                nc.vector.tensor_copy(Weo[pr, cE], ps[0:32, 128:256:4])
                nc.vector.tensor_copy(Woe[pr, cE], ps[0:32, 2:128:4])
                nc.vector.tensor_copy(Woo[pr, cE], ps[0:32, 130:256:4])
                nc.vector.tensor_copy(WeO[po, cEO], ps[0:64, 256:384:4])
                nc.vector.tensor_copy(WoO[po, cEO], ps[0:64, 258:384:4])
                nc.vector.tensor_copy(WOe[pr, cO], ps[0:32, 1:128:2])
                nc.vector.tensor_copy(WOo[pr, cO], ps[0:32, 129:256:2])
                nc.vector.tensor_copy(WOO[po, cO], ps[0:64, 257:384:2])

            rep_ctx = tc.For_i(0, reps, 1) if reps > 1 else None
            if rep_ctx is not None:
                rep_ctx.__enter__()
            for ci in range(N_CHUNKS * unroll):
                r0 = (ci % N_CHUNKS) * CHUNK
                # ---- transpose-in
                xT = stagep.tile([128, FTILES * CHUNK], _BF16, tag="stage")
                for ntp in range(2):
                    xbfs = []
                    for nn in range(2):
                        nt = ntp * 2 + nn
                        xbf = xbfp.tile([128, D_FEAT], _BF16, tag="xbf")
                        nc.gpsimd.dma_start(
                            xbf[:], xs[r0 + nt * 128: r0 + (nt + 1) * 128, :])
                        xbfs.append(xbf)
                    for fc in range(FTILES):
                        ps = tpp.tile([128, 512], _BF16, tag="tp")
                        for nn in range(2):
                            nc.tensor.transpose(
                                ps[:, nn * 128:(nn + 1) * 128],
                                xbfs[nn][:, fc * 128:(fc + 1) * 128], ident[:])
                        eng = nc.vector if fc % 2 else nc.scalar
                        (eng.tensor_copy if eng is nc.vector else eng.copy)(
                            xT[:, fc * CHUNK + ntp * 256:
                               fc * CHUNK + ntp * 256 + 256],
                            ps[:, 0:256])

                def xsl(g):
                    return xT[:, g * CHUNK:(g + 1) * CHUNK]

                # ---- gconvA + L1/L2 butterflies
                # vv slots: 0..7 v+, 8..15 v-, 16..31 u-
                vv = stagep.tile([128, FTILES * CHUNK], _BF16, tag="stage")
                for tu in range(HT):
                    ps = mmp.tile([128, CHUNK], _F32, tag="mm")
                    nc.tensor.matmul(ps[:], AT[:, at_sl(_at_n(tu))],
                                     xsl(tu), start=True, stop=False)
                    nc.tensor.matmul(ps[:], AT[:, at_sl(_at_rn(31 - tu))],
                                     xsl(31 - tu), start=False, stop=True)
                    sl = slice((16 + tu) * CHUNK, (17 + tu) * CHUNK)
                    if tu % 2:
                        nc.scalar.copy(vv[:, sl], ps[:])
                    else:
                        nc.vector.tensor_copy(vv[:, sl], ps[:])
                for tv in range(8):
                    ps_p = mmp.tile([128, CHUNK], _F32, tag="mm")
                    nc.tensor.matmul(ps_p[:], AT[:, at_sl(_at_n(tv))],
                                     xsl(tv), start=True, stop=False)
                    nc.tensor.matmul(ps_p[:], AT[:, at_sl(_at_r(31 - tv))],
                                     xsl(31 - tv), start=False, stop=True)
                    ps_r = mmp.tile([128, CHUNK], _F32, tag="mm")
                    nc.tensor.matmul(ps_r[:], AT[:, at_sl(_at_r(15 - tv))],
                                     xsl(15 - tv), start=True, stop=False)
                    nc.tensor.matmul(ps_r[:], AT[:, at_sl(_at_n(16 + tv))],
                                     xsl(16 + tv), start=False, stop=True)
                    r1 = r1p.tile([128, CHUNK], _BF16, tag="r1")
                    nc.scalar.copy(r1[:], ps_r[:])
                    nc.vector.tensor_add(
                        vv[:, tv * CHUNK:(tv + 1) * CHUNK], ps_p[:], r1[:])
                    nc.vector.tensor_sub(
                        vv[:, (8 + tv) * CHUNK:(9 + tv) * CHUNK],
                        ps_p[:], r1[:])

                def vsl(s):
                    return vv[:, s * CHUNK:(s + 1) * CHUNK]

                # ---- fwd dense: z2 slots 0..7 Ee, 8..15 Eo, 16..31 O
                z2 = stagep.tile([128, FTILES * CHUNK], _BF16, tag="stage")
                for tau in range(HT):
                    fwh = []
                    for hh in range(2):
                        fw = fwp.tile([128, 8, 128], _BF16, tag="fw")
                        nc.sync.dma_start(
                            fw[:], fwdw_o[tau][:, hh * 8:(hh + 1) * 8, :])
                        fwh.append(fw)
                    ps = mmp.tile([128, CHUNK], _F32, tag="mm")
                    for fc in range(HT):
                        nc.tensor.matmul(ps[:], fwh[fc // 8][:, fc % 8, :],
                                         vsl(16 + fc),
                                         start=(fc == 0), stop=(fc == HT - 1))
                    sl = slice((16 + tau) * CHUNK, (17 + tau) * CHUNK)
                    if tau % 2:
                        nc.scalar.copy(z2[:, sl], ps[:])
                    else:
                        nc.vector.tensor_copy(z2[:, sl], ps[:])
                for side in range(2):
                    for c in range(8):
                        fwe = fwp.tile([128, 8, 128], _BF16, tag="fw")
                        nc.sync.dma_start(fwe[:], fwdw_e[side, c])
                        ps = mmp.tile([128, CHUNK], _F32, tag="mm")
                        for kc in range(8):
                            nc.tensor.matmul(ps[:], fwe[:, kc, :],
                                             vsl(8 * side + kc),
                                             start=(kc == 0), stop=(kc == 7))
                        sl = slice((8 * side + c) * CHUNK,
                                   (8 * side + c + 1) * CHUNK)
                        if c % 2:
                            nc.scalar.copy(z2[:, sl], ps[:])
                        else:
                            nc.vector.tensor_copy(z2[:, sl], ps[:])

                def z2sl(s):
                    return z2[:, s * CHUNK:(s + 1) * CHUNK]

                # ---- gconvD in (Ee, Eo, O) basis
                z3 = stagep.tile([128, FTILES * CHUNK], _BF16, tag="stage")
                for c in range(8):
                    for oi, (w_d, w_x, w_o) in enumerate(
                            ((Wee, Weo, WeO), (Woe, Woo, WoO))):
                        ps = mmp.tile([128, CHUNK], _F32, tag="mm")
                        nc.tensor.matmul(
                            ps[:], w_d[:, 128 * c:128 * c + 128], z2sl(c),
                            start=True, stop=False)
                        nc.tensor.matmul(
                            ps[:], w_o[:, 256 * c:256 * c + 128],
                            z2sl(16 + 2 * c), start=False, stop=False)
                        nc.tensor.matmul(
                            ps[:], w_o[:, 256 * c + 128:256 * c + 256],
                            z2sl(16 + 2 * c + 1), start=False, stop=False)
                        nc.tensor.matmul(
                            ps[:], w_x[:, 128 * c:128 * c + 128], z2sl(8 + c),
                            start=False, stop=True)
                        sl = slice((8 * oi + c) * CHUNK,
                                   (8 * oi + c + 1) * CHUNK)
                        if c % 2:
                            nc.scalar.copy(z3[:, sl], ps[:])
                        else:
                            nc.vector.tensor_copy(z3[:, sl], ps[:])
                for to in range(HT):
                    ps = mmp.tile([128, CHUNK], _F32, tag="mm")
                    nc.tensor.matmul(ps[:], WOO[:, 128 * to:128 * to + 128],
                                     z2sl(16 + to), start=True, stop=False)
                    nc.tensor.matmul(ps[:], WOe[:, 128 * to:128 * to + 128],
                                     z2sl(to // 2), start=False, stop=False)
                    nc.tensor.matmul(ps[:], WOo[:, 128 * to:128 * to + 128],
                                     z2sl(8 + to // 2), start=False, stop=True)
                    sl = slice((16 + to) * CHUNK, (17 + to) * CHUNK)
                    if to % 2:
                        nc.scalar.copy(z3[:, sl], ps[:])
                    else:
                        nc.vector.tensor_copy(z3[:, sl], ps[:])

                # ---- inverse: paired strips
                for pi, (cd, cr) in enumerate(PAIRS):
                    ive = []
                    for ab in range(2):
                        iv = ivp.tile([128, 8, QW], _BF16, tag="iv")
                        nc.sync.dma_start(iv[:], invw_e[pi, ab])
                        ive.append(iv)
                    ivo = {}
                    for c in (cd, cr):
                        halves = []
                        for hh in range(2):
                            iv = ivp.tile([128, 8, QW], _BF16, tag="iv")
                            nc.sync.dma_start(
                                iv[:], invw_o[c][:, hh * 8:(hh + 1) * 8, :])
                            halves.append(iv)
                        ivo[c] = halves
                    for nt in range(CHUNK // 128):
                        def zt(s):
                            return z3[:, s * CHUNK + nt * 128:
                                      s * CHUNK + (nt + 1) * 128]
                        ps_a = stp.tile([128, QW], _F32, tag="st")
                        for kc in range(8):
                            nc.tensor.matmul(ps_a[:], zt(kc), ive[0][:, kc, :],
                                             start=(kc == 0), stop=False)
                        nc.tensor.matmul(
                            ps_a[:], ones1[:],
                            bsa_bf[0:1, pi * QW:(pi + 1) * QW],
                            start=False, stop=True)
                        ps_b = stp.tile([128, QW], _F32, tag="st")
                        for kc in range(8):
                            nc.tensor.matmul(ps_b[:], zt(8 + kc),
                                             ive[1][:, kc, :],
                                             start=(kc == 0), stop=False)
                        nc.tensor.matmul(
                            ps_b[:], ones1[:],
                            bsb_bf[0:1, pi * QW:(pi + 1) * QW],
                            start=False, stop=True)
                        sb_s = ostp.tile([128, QW], _F32, tag="ost")
                        nc.scalar.copy(sb_s[:], ps_b[:])
                        sD = ostp.tile([128, QW], _F32, tag="ost")
                        sR = ostp.tile([128, QW], _F32, tag="ost")
                        nc.vector.tensor_add(sD[:], ps_a[:], sb_s[:])
                        nc.vector.tensor_sub(sR[:, ::-1], ps_a[:], sb_s[:])
                        for c, sS in ((cd, sD), (cr, sR)):
                            ps_t = stp.tile([128, QW], _F32, tag="st")
                            for kc in range(HT):
                                nc.tensor.matmul(
                                    ps_t[:], zt(16 + kc),
                                    ivo[c][kc // 8][:, kc % 8, :],
                                    start=(kc == 0), stop=False)
                            nc.tensor.matmul(
                                ps_t[:], ones1[:],
                                bt_bf[0:1, c * QW:(c + 1) * QW],
                                start=False, stop=True)
                            op = ostp.tile([128, QW], _F32, tag="ost")
                            om = ostp.tile([128, QW], _F32, tag="ost")
                            nc.vector.tensor_add(op[:], sS[:], ps_t[:])
                            nc.vector.tensor_sub(om[:, ::-1], sS[:], ps_t[:])
                            rows = slice(r0 + nt * 128, r0 + (nt + 1) * 128)
                            nc.sync.dma_start(
                                out[rows, PLUS_START[c]:PLUS_START[c] + QW],
                                op[:])
                            nc.sync.dma_start(
                                out[rows, MINUS_LO[c]:MINUS_LO[c] + QW],
                                om[:])
            if rep_ctx is not None:
                rep_ctx.__exit__(None, None, None)
    nc.finalize()
    return nc


_CACHE = {}


def kernel(x, A, D, bias):
    if "nc" not in _CACHE:
        _CACHE["consts"] = _host_constants()
        _CACHE["nc"] = _build_program()
    nc = _CACHE["nc"]
    fwdw_o, fwdw_e, invw_o, invw_e, out_plus, out_minus = _CACHE["consts"]
    bsa, bsb, bt = _host_bias(bias, out_plus, out_minus)

    x = np.ascontiguousarray(x, dtype=np.float32)
    in_maps = []
    for c in range(N_CORES):
        in_maps.append({
            "xs": x[c * N_SHARD:(c + 1) * N_SHARD],
            "Aw": np.ascontiguousarray(A, dtype=np.float32),
            "Dw": np.ascontiguousarray(D, dtype=np.float32),
            "bias_sa": bsa, "bias_sb": bsb, "bias_t": bt,
            "fwdw_o": fwdw_o, "fwdw_e": fwdw_e,
            "invw_o": invw_o, "invw_e": invw_e,
        })
    res = run_bass_kernel_spmd(nc, in_maps, core_ids=list(range(N_CORES)))
    return np.concatenate([res.results[c]["out"] for c in range(N_CORES)],
                          axis=0)
